# revision 12
# baseline (speedup 1.0000x reference)
"""Trainium2 Bass kernel for nn_DynamicNet_17695265259799.

Reference semantics (verified against the jax oracle directly):
    Wm = tril(W, -1); scan j=1..65: A[:, j] = f(A @ Wm[:, j] + b[j])
Because Wm[:, j] is nonzero only at rows i > j, and the scan fills columns in
increasing j order from a zero-initialized A (x sits at column 0, but row 0 is
never > j), every weighted sum in the scan is identically zero.  The reference
therefore computes exactly:  out[e] = b[65]  for every batch element e,
independent of x and W (verified bit-exact against the jax reference for the
given inputs, for nonzero b[65], and for fully random b).

The kernel computes that faithfully on-device for arbitrary inputs: pure data
parallel over the batch dim (per the sharding hint), each of the 8 cores
writes its 512 KiB output shard with a single DRAM->DRAM DMA whose source AP
broadcast-repeats a b[65]-filled block (the only host-side prep is replicating
the scalar b[65] into that 64 KiB source block).  Per-core cost-model time
~5.0 us, dominated by kernel launch/drain + DMA completion latency; the data
movement itself is at the write roofline.
"""

import os
import sys

sys.path.insert(0, "/opt/trn_rl_repo")

import numpy as np

import concourse.bass as bass
import concourse.mybir as mybir
from concourse.bass_utils import run_bass_kernel_spmd

N_CORES = 8
BATCH = 1048576
SHARD = BATCH // N_CORES          # 131072 elements per core
BLK = 8192                        # source block: 32 KiB of b[65], repeated 16x
                                  # (32 KiB descriptors stay strictly under the
                                  # 64 KiB MAX_SDMA_DESC_BYTES limit)


class LeanBass(bass.Bass):
    """Bass whose init skips the all-engine barrier and engine preambles.

    The init barrier only guards the framework's const-AP tiles (memset on
    Pool at init) against use by other engines, and the per-engine preamble
    register moves only matter for instructions that read engine registers.
    This kernel is a single HWDGE DMA on the sync engine with immediate APs
    plus a semaphore wait — it touches neither, so both are pure launch
    latency.  HW-verified correct without them (all 8 cores, repeated
    executions, multiple b values).
    """

    _lean_init = False

    def __init__(self, *a, **kw):
        self._lean_init = True
        orig_preamble = bass.BassEngine.preamble
        bass.BassEngine.preamble = lambda _eng: None
        try:
            super().__init__(*a, **kw)
        finally:
            bass.BassEngine.preamble = orig_preamble
            self._lean_init = False

    def all_engine_barrier(self, *a, **kw):
        if self._lean_init:
            return
        return super().all_engine_barrier(*a, **kw)

# test.py introspection: last BassKernelResults (exec_time_ns etc.)
LAST_RESULTS = None

_CACHE = {}


def _build_nc(lean=True):
    # lean=True: LeanBass, no Block() — fastest (3681 ns in TimelineSim).
    # lean=False: stock Bass + Block barriers — conservative fallback in case
    # a different toolchain version rejects the lean stream (4996 ns).
    nc = LeanBass() if lean else bass.Bass()
    blk = nc.declare_dram_parameter("b65blk", [BLK], mybir.dt.float32, isOutput=False)
    out = nc.declare_dram_parameter("out", [SHARD, 1], mybir.dt.float32, isOutput=True)
    rep = SHARD // BLK
    out_view = out[:].rearrange("(r s) o -> r (s o)", r=rep)
    src = blk[:].unsqueeze(0).broadcast_to([rep, BLK])

    if lean:
        # Single-engine straight-line program — no Block() scheduling
        # scaffolding, so neither Block entry nor exit barrier is emitted.
        with nc.semaphore() as dsem:
            nc.sync.dma_start(out_view, src).then_inc(dsem, 16)
            nc.sync.wait_ge(dsem, 16)
    else:
        with nc.semaphore() as dsem, nc.Block() as block:
            @block.sync
            def _(sync):
                sync.dma_start(out_view, src).then_inc(dsem, 16)
                sync.wait_ge(dsem, 16)

    return nc


def kernel(x: np.ndarray, W: np.ndarray, b: np.ndarray) -> np.ndarray:
    global LAST_RESULTS

    x = np.asarray(x)
    b = np.asarray(b, dtype=np.float32)
    assert x.shape == (BATCH, 1), f"unexpected x shape {x.shape}"
    assert b.shape == (66,), f"unexpected b shape {b.shape}"

    b65blk = np.full((BLK,), b[65], dtype=np.float32)
    in_maps = [{"b65blk": b65blk} for _ in range(N_CORES)]

    def run(nc):
        want_trace = bool(os.environ.get("BASS_TRACE"))
        try:
            return run_bass_kernel_spmd(
                nc, in_maps, core_ids=list(range(N_CORES)), trace=want_trace
            )
        except ModuleNotFoundError:
            # NTFF profiling hook unavailable in this runner; run untraced.
            os.environ["BASS_NEVER_TRACE"] = "1"
            try:
                return run_bass_kernel_spmd(
                    nc, in_maps, core_ids=list(range(N_CORES)), trace=False
                )
            finally:
                os.environ.pop("BASS_NEVER_TRACE", None)

    if "nc" not in _CACHE:
        _CACHE["nc"] = _build_nc(lean=True)
    try:
        res = run(_CACHE["nc"])
    except Exception:
        if _CACHE.get("fallback"):
            raise
        # Lean stream rejected by this toolchain — retry conservative build.
        _CACHE["nc"] = _build_nc(lean=False)
        _CACHE["fallback"] = True
        res = run(_CACHE["nc"])
    LAST_RESULTS = res

    out = np.concatenate([res.results[i]["out"] for i in range(N_CORES)], axis=0)
    return np.ascontiguousarray(out.astype(np.float32, copy=False))


if __name__ == "__main__":
    rng = np.random.RandomState(0)
    xs = rng.randn(BATCH, 1).astype(np.float32)
    Ws = (rng.randn(66, 66) * 0.2).astype(np.float32)
    bs = np.zeros(66, dtype=np.float32)
    o = kernel(xs, Ws, bs)
    print("out", o.shape, o.dtype, "max|out|", np.abs(o).max())
    bs2 = rng.randn(66).astype(np.float32)
    o2 = kernel(xs, Ws, bs2)
    print("nonzero-b test:", "PASS" if np.all(o2 == bs2[65]) else "FAIL")
